# revision 12
# baseline (speedup 1.0000x reference)
"""Trainium2 Bass kernel for EuclideanSimilarity (retrieval_knn).

Reference computation per batch b (B=8, L=4096, D=128):
    projected = x @ W.T + b                      [L, D]
    q = avgpool2(x) @ W.T + b                    [L/2, D]   (== avgpool2(projected))
    power = ||q_i||^2 + ||k_j||^2 - 2 q_i.k_j    [L/2, L]
    sim = exp(-sqrt(max(power, 0)))
    k = sim @ projected                          [L/2, D]
    returns (q, k, v=k)

Sharding: data-parallel over batch, one batch element per NeuronCore (8 cores).
All device tensors keep the feature dim D=128 on SBUF partitions where the
matmuls contract over it; host pre-transposes x and post-transposes q/k
(host-side layout prep is free wrt HW exec time).
"""

import os
import sys

for _p in ("/opt/trn_rl_repo", "/root/.axon_site/_ro/trn_rl_repo"):
    if os.path.isdir(_p) and _p not in sys.path:
        sys.path.insert(0, _p)

import numpy as np

import concourse.bass as bass
import concourse.mybir as mybir
from concourse import bacc
from concourse.bass_utils import run_bass_kernel_spmd
from concourse.tile import TileContext

B, L, D = 8, 4096, 128
LQ = L // 2          # 2048 pooled queries
P = 128              # partitions
NI = 512             # i-chunk (queries per chunk)
NCHUNK = LQ // NI    # 4
NJT = L // P         # 32 j-tiles
F32 = mybir.dt.float32

AF = mybir.ActivationFunctionType


def build_nc():
    nc = bacc.Bacc("TRN2", target_bir_lowering=False)

    xT = nc.declare_dram_parameter("xT", [P, L], F32, isOutput=False)
    WT = nc.declare_dram_parameter("WT", [P, D], F32, isOutput=False)       # W.T
    Wm2T = nc.declare_dram_parameter("Wm2T", [P, D], F32, isOutput=False)   # (-2W).T
    WhT = nc.declare_dram_parameter("WhT", [P, D], F32, isOutput=False)     # (0.5W).T
    bcols = nc.declare_dram_parameter("bcols", [P, 2], F32, isOutput=False)  # [b, -2b]
    b_bcast_in = nc.declare_dram_parameter("b_bcast", [P, D], F32, isOutput=False)
    ones_in = nc.declare_dram_parameter("ones_mat", [P, P], F32, isOutput=False)

    qT_out = nc.declare_dram_parameter("qT", [P, LQ], F32, isOutput=True)
    kT_out = nc.declare_dram_parameter("kT", [P, LQ], F32, isOutput=True)

    with TileContext(nc) as tc:
        with (
            tc.tile_pool(name="consts", bufs=1) as consts,
            tc.tile_pool(name="big", bufs=1) as big,
            tc.tile_pool(name="stripp", bufs=1) as stripp,
            tc.tile_pool(name="work", bufs=2) as work,
            tc.tile_pool(name="ps1", bufs=3, space="PSUM") as ps1,
            tc.tile_pool(name="psqk", bufs=3, space="PSUM") as psqk,
            tc.tile_pool(name="psk", bufs=2, space="PSUM") as psk,
        ):
            # ---- constants ----
            WT_sb = consts.tile([P, D], F32)
            Wm2T_sb = consts.tile([P, D], F32)
            WhT_sb = consts.tile([P, D], F32)
            bcols_sb = consts.tile([P, 2], F32)
            b_bcast = consts.tile([P, D], F32)
            ones_sb = consts.tile([P, P], F32)
            nc.sync.dma_start(out=WT_sb[:], in_=WT[:])
            nc.sync.dma_start(out=Wm2T_sb[:], in_=Wm2T[:])
            nc.sync.dma_start(out=WhT_sb[:], in_=WhT[:])
            nc.sync.dma_start(out=bcols_sb[:], in_=bcols[:])
            nc.sync.dma_start(out=b_bcast[:], in_=b_bcast_in[:])
            nc.sync.dma_start(out=ones_sb[:], in_=ones_in[:])
            b_col = bcols_sb[:, 0:1]
            bm2_col = bcols_sb[:, 1:2]

            # ---- x load + pooling ----
            xT_sb = big.tile([P, L], F32)
            nc.sync.dma_start(out=xT_sb[:], in_=xT[:])
            xT_pairs = xT_sb.rearrange("p (i two) -> p i two", two=2)
            xpool = big.tile([P, LQ], F32)  # xT[:, 2i] + xT[:, 2i+1]
            nc.vector.tensor_add(xpool[:], xT_pairs[:, :, 0], xT_pairs[:, :, 1])

            # ---- projT_m2[e, l] = -2 * (W x + b)^T ----
            projTm2 = big.tile([P, L], F32)
            for c in range(L // NI):
                ps = ps1.tile([P, NI], F32, tag="ps1")
                nc.tensor.matmul(
                    ps, Wm2T_sb[:], xT_sb[:, c * NI:(c + 1) * NI],
                    start=True, stop=True,
                )
                nc.scalar.add(projTm2[:, c * NI:(c + 1) * NI], ps, bm2_col)

            # ---- proj_nat tiles [l(128), e] and ksq ----
            projnat = big.tile([P, L], F32)  # 32 tiles of [128, 128] along free
            ksq = consts.tile([P, NJT], F32)
            sq_scratch = work.tile([P, D], F32, tag="sqs")
            for t in range(NJT):
                ps = ps1.tile([P, D], F32, tag="ps1")
                nc.tensor.matmul(
                    ps, xT_sb[:, t * P:(t + 1) * P], WT_sb[:],
                    start=True, stop=True,
                )
                seg = projnat[:, t * P:(t + 1) * P]
                nc.vector.tensor_add(seg, ps, b_bcast[:])
                # ksq[:, t] = sum_e seg^2  (ACT Square with free-dim accumulator)
                sq_scratch = work.tile([P, D], F32, tag="sqs")
                nc.scalar.activation(
                    sq_scratch[:], seg, AF.Square, accum_out=ksq[:, t:t + 1]
                )

            # ---- qT[e, i] = 0.5*W @ xpool + b ----
            qT_sb = big.tile([P, LQ], F32)
            for c in range(NCHUNK):
                ps = ps1.tile([P, NI], F32, tag="ps1")
                nc.tensor.matmul(
                    ps, WhT_sb[:], xpool[:, c * NI:(c + 1) * NI],
                    start=True, stop=True,
                )
                nc.scalar.add(qT_sb[:, c * NI:(c + 1) * NI], ps, b_col)
            nc.sync.dma_start(out=qT_out[:], in_=qT_sb[:])

            # ---- qsq_bcast[p, i] = ||q_i||^2 for all partitions ----
            # ones[e, p] stationary: out[p, i] = sum_e sq_qT[e, i] -> reduce +
            # partition-broadcast in a single matmul.
            sq_qT = big.tile([P, LQ], F32)
            nc.vector.tensor_mul(sq_qT[:], qT_sb[:], qT_sb[:])
            qsq_bcast = big.tile([P, LQ], F32)
            for c in range(NCHUNK):
                ps = ps1.tile([P, NI], F32, tag="ps1")
                nc.tensor.matmul(
                    ps, ones_sb[:], sq_qT[:, c * NI:(c + 1) * NI],
                    start=True, stop=True,
                )
                nc.vector.tensor_copy(qsq_bcast[:, c * NI:(c + 1) * NI], ps)

            # ---- main loop over query chunks ----
            kT_sb = big.tile([P, LQ], F32)
            for c in range(NCHUNK):
                strip = stripp.tile([P, NJT * NI], F32, tag="strip")
                qs = qsq_bcast[:, c * NI:(c + 1) * NI]
                qchunk = qT_sb[:, c * NI:(c + 1) * NI]
                for jt in range(NJT):
                    ps2 = psqk.tile([P, NI], F32, tag="qk")
                    nc.tensor.matmul(
                        ps2, projTm2[:, jt * P:(jt + 1) * P], qchunk,
                        start=True, stop=True,
                    )
                    # power = (-2qk) * 1 + ksq[j]  + qsq[i]
                    nc.vector.affine_then_add(
                        strip[:, jt * NI:(jt + 1) * NI], ps2, qs,
                        scale=1.0, bias=ksq[:, jt:jt + 1],
                    )
                nc.scalar.activation(strip[:], strip[:], AF.Sqrt)
                nc.scalar.activation(strip[:], strip[:], AF.Exp, scale=-1.0)
                ps3 = psk.tile([P, NI], F32, tag="kacc")
                for jt in range(NJT):
                    nc.tensor.matmul(
                        ps3, projnat[:, jt * P:(jt + 1) * P],
                        strip[:, jt * NI:(jt + 1) * NI],
                        start=(jt == 0), stop=(jt == NJT - 1),
                    )
                nc.vector.tensor_copy(kT_sb[:, c * NI:(c + 1) * NI], ps3)
                nc.sync.dma_start(
                    out=kT_out[:, c * NI:(c + 1) * NI],
                    in_=kT_sb[:, c * NI:(c + 1) * NI],
                )

    nc.compile()
    return nc


_NC_CACHE = {}


def _get_nc():
    if "nc" not in _NC_CACHE:
        _NC_CACHE["nc"] = build_nc()
    return _NC_CACHE["nc"]


def kernel(x, W, b):
    x = np.asarray(x, dtype=np.float32)
    W = np.asarray(W, dtype=np.float32)
    b = np.asarray(b, dtype=np.float32)

    nc = _get_nc()

    WT = np.ascontiguousarray(W.T)
    Wm2T = np.ascontiguousarray((-2.0 * W).T)
    WhT = np.ascontiguousarray((0.5 * W).T)
    bcols = np.stack([b, -2.0 * b], axis=1).astype(np.float32)
    b_bcast = np.broadcast_to(b.reshape(1, D), (P, D)).astype(np.float32)
    b_bcast = np.ascontiguousarray(b_bcast)
    ones_mat = np.ones((P, P), np.float32)

    in_maps = []
    for i in range(B):
        in_maps.append({
            "xT": np.ascontiguousarray(x[i].T),
            "WT": WT,
            "Wm2T": Wm2T,
            "WhT": WhT,
            "bcols": bcols,
            "b_bcast": b_bcast,
            "ones_mat": ones_mat,
        })

    trace = bool(int(os.environ.get("KBENCH_TRACE", "0")))
    kres = run_bass_kernel_spmd(nc, in_maps, list(range(B)), trace=trace)
    _NC_CACHE["last_result"] = kres
    res = kres.results

    q = np.stack([np.ascontiguousarray(r["qT"].T) for r in res])
    k = np.stack([np.ascontiguousarray(r["kT"].T) for r in res])
    return q, k, k


# revision 14
# speedup vs baseline: 20.0643x; 20.0643x over previous
"""Trainium2 Bass kernel for EuclideanSimilarity (retrieval_knn).

Reference computation per batch b (B=8, L=4096, D=128):
    projected = x @ W.T + b                      [L, D]
    q = avgpool2(x) @ W.T + b                    [L/2, D]   (== avgpool2(projected))
    power = ||q_i||^2 + ||k_j||^2 - 2 q_i.k_j    [L/2, L]
    sim = exp(-sqrt(max(power, 0)))
    k = sim @ projected                          [L/2, D]
    returns (q, k, v=k)

Sharding: data-parallel over batch, one batch element per NeuronCore (8 cores).
All device tensors keep the feature dim D=128 on SBUF partitions where the
matmuls contract over it; host pre-transposes x and post-transposes q/k
(host-side layout prep is free wrt HW exec time).
"""

import os
import sys

for _p in ("/opt/trn_rl_repo", "/root/.axon_site/_ro/trn_rl_repo"):
    if os.path.isdir(_p) and _p not in sys.path:
        sys.path.insert(0, _p)

import numpy as np

import concourse.bass as bass
import concourse.mybir as mybir
from concourse import bacc
from concourse.bass_utils import run_bass_kernel_spmd
from concourse.tile import TileContext

B, L, D = 8, 4096, 128
LQ = L // 2          # 2048 pooled queries
P = 128              # partitions
NI = 512             # i-chunk (queries per chunk)
NCHUNK = LQ // NI    # 4
NJT = L // P         # 32 j-tiles
F32 = mybir.dt.float32

AF = mybir.ActivationFunctionType


def build_nc(repeat=1):
    nc = bacc.Bacc("TRN2", target_bir_lowering=False)

    xT = nc.declare_dram_parameter("xT", [P, L], F32, isOutput=False)
    WT = nc.declare_dram_parameter("WT", [P, D], F32, isOutput=False)       # W.T
    Wm2T = nc.declare_dram_parameter("Wm2T", [P, D], F32, isOutput=False)   # (-2W).T
    WhT = nc.declare_dram_parameter("WhT", [P, D], F32, isOutput=False)     # (0.5W).T
    bcols = nc.declare_dram_parameter("bcols", [P, 2], F32, isOutput=False)  # [b, -2b]
    b_bcast_in = nc.declare_dram_parameter("b_bcast", [P, D], F32, isOutput=False)
    ones_in = nc.declare_dram_parameter("ones_mat", [P, P], F32, isOutput=False)

    qT_out = nc.declare_dram_parameter("qT", [P, LQ], F32, isOutput=True)
    kT_out = nc.declare_dram_parameter("kT", [P, LQ], F32, isOutput=True)

    with TileContext(nc) as tc:
      for _rep in range(repeat):
        with (
            tc.tile_pool(name="consts", bufs=1) as consts,
            tc.tile_pool(name="big", bufs=1) as big,
            tc.tile_pool(name="stripp", bufs=1) as stripp,
            tc.tile_pool(name="work", bufs=2) as work,
            tc.tile_pool(name="ps1", bufs=3, space="PSUM") as ps1,
            tc.tile_pool(name="psqk", bufs=3, space="PSUM") as psqk,
            tc.tile_pool(name="psk", bufs=2, space="PSUM") as psk,
        ):
            # ---- constants ----
            WT_sb = consts.tile([P, D], F32)
            Wm2T_sb = consts.tile([P, D], F32)
            WhT_sb = consts.tile([P, D], F32)
            bcols_sb = consts.tile([P, 2], F32)
            b_bcast = consts.tile([P, D], F32)
            ones_sb = consts.tile([P, P], F32)
            nc.sync.dma_start(out=WT_sb[:], in_=WT[:])
            nc.sync.dma_start(out=Wm2T_sb[:], in_=Wm2T[:])
            nc.sync.dma_start(out=WhT_sb[:], in_=WhT[:])
            nc.sync.dma_start(out=bcols_sb[:], in_=bcols[:])
            nc.sync.dma_start(out=b_bcast[:], in_=b_bcast_in[:])
            nc.sync.dma_start(out=ones_sb[:], in_=ones_in[:])
            b_col = bcols_sb[:, 0:1]
            bm2_col = bcols_sb[:, 1:2]

            # ---- x load + pooling ----
            xT_sb = big.tile([P, L], F32)
            nc.sync.dma_start(out=xT_sb[:], in_=xT[:])
            xT_pairs = xT_sb.rearrange("p (i two) -> p i two", two=2)
            xpool = big.tile([P, LQ], F32)  # xT[:, 2i] + xT[:, 2i+1]
            nc.vector.tensor_add(xpool[:], xT_pairs[:, :, 0], xT_pairs[:, :, 1])

            # ---- projT_m2[e, l] = -2 * (W x + b)^T ----
            projTm2 = big.tile([P, L], F32)
            for c in range(L // NI):
                ps = ps1.tile([P, NI], F32, tag="ps1")
                nc.tensor.matmul(
                    ps, Wm2T_sb[:], xT_sb[:, c * NI:(c + 1) * NI],
                    start=True, stop=True,
                )
                nc.scalar.add(projTm2[:, c * NI:(c + 1) * NI], ps, bm2_col)

            # ---- proj_nat tiles [l(128), e] and ksq ----
            projnat = big.tile([P, L], F32)  # 32 tiles of [128, 128] along free
            ksq = consts.tile([P, NJT], F32)
            sq_scratch = work.tile([P, D], F32, tag="sqs")
            for t in range(NJT):
                ps = ps1.tile([P, D], F32, tag="ps1")
                nc.tensor.matmul(
                    ps, xT_sb[:, t * P:(t + 1) * P], WT_sb[:],
                    start=True, stop=True,
                )
                seg = projnat[:, t * P:(t + 1) * P]
                nc.vector.tensor_add(seg, ps, b_bcast[:])
                # ksq[:, t] = sum_e seg^2  (ACT Square with free-dim accumulator)
                sq_scratch = work.tile([P, D], F32, tag="sqs")
                nc.scalar.activation(
                    sq_scratch[:], seg, AF.Square, accum_out=ksq[:, t:t + 1]
                )

            # ---- qT[e, i] = 0.5*W @ xpool + b ----
            qT_sb = big.tile([P, LQ], F32)
            for c in range(NCHUNK):
                ps = ps1.tile([P, NI], F32, tag="ps1")
                nc.tensor.matmul(
                    ps, WhT_sb[:], xpool[:, c * NI:(c + 1) * NI],
                    start=True, stop=True,
                )
                nc.scalar.add(qT_sb[:, c * NI:(c + 1) * NI], ps, b_col)
            nc.sync.dma_start(out=qT_out[:], in_=qT_sb[:])

            # ---- qsq_bcast[p, i] = ||q_i||^2 for all partitions ----
            # ones[e, p] stationary: out[p, i] = sum_e sq_qT[e, i] -> reduce +
            # partition-broadcast in a single matmul.
            sq_qT = big.tile([P, LQ], F32)
            nc.vector.tensor_mul(sq_qT[:], qT_sb[:], qT_sb[:])
            qsq_bcast = big.tile([P, LQ], F32)
            for c in range(NCHUNK):
                ps = ps1.tile([P, NI], F32, tag="ps1")
                nc.tensor.matmul(
                    ps, ones_sb[:], sq_qT[:, c * NI:(c + 1) * NI],
                    start=True, stop=True,
                )
                nc.vector.tensor_copy(qsq_bcast[:, c * NI:(c + 1) * NI], ps)

            # ---- main loop over query chunks ----
            kT_sb = big.tile([P, LQ], F32)
            for c in range(NCHUNK):
                strip = stripp.tile([P, NJT * NI], F32, tag="strip")
                qs = qsq_bcast[:, c * NI:(c + 1) * NI]
                qchunk = qT_sb[:, c * NI:(c + 1) * NI]
                for jt in range(NJT):
                    ps2 = psqk.tile([P, NI], F32, tag="qk")
                    nc.tensor.matmul(
                        ps2, projTm2[:, jt * P:(jt + 1) * P], qchunk,
                        start=True, stop=True,
                    )
                    # power = (-2qk) * 1 + ksq[j]  + qsq[i]
                    nc.vector.affine_then_add(
                        strip[:, jt * NI:(jt + 1) * NI], ps2, qs,
                        scale=1.0, bias=ksq[:, jt:jt + 1],
                    )
                nc.scalar.activation(strip[:], strip[:], AF.Sqrt)
                nc.scalar.activation(strip[:], strip[:], AF.Exp, scale=-1.0)
                ps3 = psk.tile([P, NI], F32, tag="kacc")
                for jt in range(NJT):
                    nc.tensor.matmul(
                        ps3, projnat[:, jt * P:(jt + 1) * P],
                        strip[:, jt * NI:(jt + 1) * NI],
                        start=(jt == 0), stop=(jt == NJT - 1),
                    )
                nc.vector.tensor_copy(kT_sb[:, c * NI:(c + 1) * NI], ps3)
                nc.sync.dma_start(
                    out=kT_out[:, c * NI:(c + 1) * NI],
                    in_=kT_sb[:, c * NI:(c + 1) * NI],
                )

    nc.compile()
    return nc


_NC_CACHE = {}


def _get_nc():
    if "nc" not in _NC_CACHE:
        _NC_CACHE["nc"] = build_nc()
    return _NC_CACHE["nc"]


def kernel(x, W, b):
    x = np.asarray(x, dtype=np.float32)
    W = np.asarray(W, dtype=np.float32)
    b = np.asarray(b, dtype=np.float32)

    nc = _get_nc()

    WT = np.ascontiguousarray(W.T)
    Wm2T = np.ascontiguousarray((-2.0 * W).T)
    WhT = np.ascontiguousarray((0.5 * W).T)
    bcols = np.stack([b, -2.0 * b], axis=1).astype(np.float32)
    b_bcast = np.broadcast_to(b.reshape(1, D), (P, D)).astype(np.float32)
    b_bcast = np.ascontiguousarray(b_bcast)
    ones_mat = np.ones((P, P), np.float32)

    in_maps = []
    for i in range(B):
        in_maps.append({
            "xT": np.ascontiguousarray(x[i].T),
            "WT": WT,
            "Wm2T": Wm2T,
            "WhT": WhT,
            "bcols": bcols,
            "b_bcast": b_bcast,
            "ones_mat": ones_mat,
        })

    trace = bool(int(os.environ.get("KBENCH_TRACE", "0")))
    kres = run_bass_kernel_spmd(nc, in_maps, list(range(B)), trace=trace)
    _NC_CACHE["last_result"] = kres
    res = kres.results

    q = np.stack([np.ascontiguousarray(r["qT"].T) for r in res])
    k = np.stack([np.ascontiguousarray(r["kT"].T) for r in res])
    return q, k, k
